# revision 20
# baseline (speedup 1.0000x reference)
"""AffineFlow Trainium2 kernel.

Computes out[t, n, i] = sum_j expm(t_k * A)[i, j] * x_pad[n, j] for the
4x4 homogeneous A built from (M, M0, b), with x_pad[:, 3] = 0 and the
homogeneous output row dropped.  Equivalently out[t] = x @ B_t^T with
B_t = expm(t_k * A)[:3, :3].

Distribution: data-parallel over the N=500000 points axis across 8
NeuronCores.  The tiny per-t 3x3 maps B_t are computed host-side (64
4x4 expm's, microseconds) and replicated to every core; each core
computes the 192 x 62976 result for its 62500-point shard on the
TensorEngine and streams the 48 MB to HBM in a planar (t,i)-major
layout (the only layout the PE/PSUM path can produce without an
on-chip transpose).  The host gather permutes each shard back to the
(t, n, i) layout while unsharding.

Precision: fp32 matmul on trn2 PE runs in a 2x-slower LOW_HIGH
emulation with the fp32 moving operand streaming at ~2.5 cycles/col.
Instead we split both operands into fp16 hi+lo pairs (exact to ~2^-22)
and exploit the tiny contraction dim: the stationary is the K=12 block
[Bh; Bl; Bh; Bl] against the moving block [xh; xh; xl; xl], so a
single fp16-rate stream per chunk accumulates the exact expansion
Bh xh + Bl xh + Bh xl + Bl xl = B x in fp32 PSUM.  Max abs error vs
the fp32 reference is ~1e-6.
"""

import os
import sys

import numpy as np

# ---------------------------------------------------------------------------
# Problem constants (hardcoded per the self-containment contract)
# ---------------------------------------------------------------------------
N_POINTS = 500000
T_STEPS = 64
M_SIZE = 3
N_CORES = 8
N_SHARD = N_POINTS // N_CORES  # 62500
CHUNK = 512  # one PSUM bank of fp32
N_CHUNKS = 123  # ceil(62500 / 512) -> padded shard of 62976 points
N_PAD = N_CHUNKS * CHUNK  # 62976
ROWS = T_STEPS * M_SIZE  # 192 output rows, split 128 ("A") + 64 ("B")
ROWS_A = 128
ROWS_B = ROWS - ROWS_A  # 64; two chunks packed per 128-partition tile
XBATCH = 8  # chunks per x-load / out-store DMA


def _install_ntff_hook_shim():
    """antenv.axon_hooks is missing in this image; trace=True (or a
    BASS_TRACE=1 env from a harness) would crash run_bass_kernel_spmd
    without it.  Register the hook via the boot module's ctypes factory."""
    try:
        import antenv.axon_hooks  # noqa: F401

        return
    except ImportError:
        pass
    import types

    mod = types.ModuleType("antenv.axon_hooks")
    mod._hook = None
    mod.set_axon_ntff_profile_hook = lambda h: setattr(mod, "_hook", h)
    mod.get_axon_ntff_profile_hook = lambda: mod._hook
    sys.modules["antenv.axon_hooks"] = mod
    try:
        import antenv

        antenv.axon_hooks = mod
    except ImportError:
        pass
    try:
        from trn_agent_boot.trn_boot import _ntff_profile_via_ctypes

        hook = _ntff_profile_via_ctypes("/opt/axon/libaxon_pjrt.so")
        if hook is not None:
            mod._hook = hook
    except Exception:
        pass


def _expm(mat: np.ndarray) -> np.ndarray:
    """Matrix exponential via scaling-and-squaring + Taylor (float64).
    Exact to machine precision for the tiny well-conditioned matrices here."""
    mat = mat.astype(np.float64)
    nrm = np.linalg.norm(mat, 1)
    s = max(0, int(np.ceil(np.log2(max(nrm, 1e-300) / 0.25)))) if nrm > 0.25 else 0
    a = mat / (2.0**s)
    n = mat.shape[0]
    out = np.eye(n) + a
    term = a.copy()
    for k in range(2, 24):
        term = term @ a / k
        out += term
        if np.abs(term).max() < 1e-18:
            break
    for _ in range(s):
        out = out @ out
    return out


def _build_bhl(t: np.ndarray, M: np.ndarray, M0: np.ndarray, b: np.ndarray):
    """Stationary layout: bhl[j, h*96 + tl*3 + i] = B[h*32+tl, i, j] split
    into an fp16 hi/lo pair stacked along K -> shape (6, 192) fp16."""
    A = np.zeros((M_SIZE + 1, M_SIZE + 1), dtype=np.float64)
    A[:M_SIZE, :M_SIZE] = M.astype(np.float64) + M0.astype(np.float64)
    A[:M_SIZE, M_SIZE] = b.astype(np.float64)
    B = np.stack([_expm(float(tk) * A)[:M_SIZE, :M_SIZE] for tk in t])  # (64,3,3)
    bt = B.transpose(2, 0, 1).reshape(M_SIZE, T_STEPS * M_SIZE)  # (3, 192) f64
    bh = bt.astype(np.float16)
    bl = (bt - bh.astype(np.float64)).astype(np.float16)
    return np.ascontiguousarray(np.concatenate([bh, bl, bh, bl], axis=0))  # (12, 192)


_COMPILED = {}


def _build_module():
    from concourse import bacc, mybir
    import concourse.tile as tile

    nc = bacc.Bacc(
        "TRN2",
        target_bir_lowering=False,
        debug=False,
        enable_asserts=False,
        num_devices=N_CORES,
    )
    f32 = mybir.dt.float32
    f16 = mybir.dt.float16
    # partition-major: rows 0-5 = [xh; xh] per j, rows 6-11 = [xl; xl]
    x6 = nc.dram_tensor(
        "x6", [4 * M_SIZE, N_CHUNKS * CHUNK], f16, kind="ExternalInput"
    )
    bhl = nc.dram_tensor("bhl", [4 * M_SIZE, ROWS], f16, kind="ExternalInput")
    out = nc.dram_tensor("out", [ROWS, N_PAD], f32, kind="ExternalOutput")

    with tile.TileContext(nc) as tc:
        with (
            tc.tile_pool(name="bt", bufs=1) as bt_pool,
            tc.tile_pool(name="x", bufs=4) as x_pool,
            tc.tile_pool(name="ps", bufs=4, space="PSUM") as ps_pool,
            tc.tile_pool(name="o", bufs=5) as o_pool,
        ):
            bhl_sb = bt_pool.tile([4 * M_SIZE, ROWS], f16)
            nc.sync.dma_start(bhl_sb[:], bhl[:])
            lhsT_a = bhl_sb[:, :ROWS_A]  # [12, 128]
            lhsT_b = bhl_sb[:, ROWS_A:]  # [12, 64]

            def copy(cnt, dst, src):
                # ACT-weighted 2:1 split (ACT is faster for wide fp32 copies)
                if cnt % 3 < 2:
                    nc.scalar.copy(dst, src)
                else:
                    nc.vector.tensor_copy(dst, src)

            cnt = 0
            # --- group A: output rows 0..127, full-partition tiles ---
            for cb in range(0, N_CHUNKS, XBATCH):
                nb = min(XBATCH, N_CHUNKS - cb)
                xq = x_pool.tile([4 * M_SIZE, XBATCH * CHUNK], f16, tag="xq")
                # x-loads go on the ACT HWDGE ring so they don't FIFO-block
                # the out-stores on the SP HWDGE ring
                nc.scalar.dma_start(
                    xq[:, : nb * CHUNK],
                    x6[:, cb * CHUNK : (cb + nb) * CHUNK],
                )
                o_sb = o_pool.tile([ROWS_A, XBATCH * CHUNK], f32, tag="osb")
                for s in range((nb + 1) // 2):
                    k0 = 2 * s
                    kn = min(2, nb - k0)  # chunks in this psum tile
                    ps = ps_pool.tile([ROWS_A, 2 * CHUNK], f32, tag="ps")
                    for k in range(kn):
                        nc.tensor.matmul(
                            ps[:, k * CHUNK : (k + 1) * CHUNK],
                            lhsT_a,
                            xq[:, (k0 + k) * CHUNK : (k0 + k + 1) * CHUNK],
                            start=True,
                            stop=True,
                        )
                    copy(
                        cnt,
                        o_sb[:, k0 * CHUNK : (k0 + kn) * CHUNK],
                        ps[:, : kn * CHUNK],
                    )
                    cnt += 1
                nc.sync.dma_start(
                    out[:ROWS_A, cb * CHUNK : (cb + nb) * CHUNK],
                    o_sb[:, : nb * CHUNK],
                )
            # --- group B: output rows 128..191; [64, .] tiles at alternating
            # partition base per block so SBUF-port load stays balanced ---
            for bi, cb in enumerate(range(0, N_CHUNKS, XBATCH)):
                nb = min(XBATCH, N_CHUNKS - cb)
                base = ROWS_B * (bi % 2)
                xq = x_pool.tile([4 * M_SIZE, XBATCH * CHUNK], f16, tag="xq")
                nc.scalar.dma_start(
                    xq[:, : nb * CHUNK],
                    x6[:, cb * CHUNK : (cb + nb) * CHUNK],
                )
                o_sb = o_pool.tile([2 * ROWS_B, XBATCH * CHUNK], f32, tag="osbB")
                for s in range((nb + 1) // 2):
                    k0 = 2 * s
                    kn = min(2, nb - k0)
                    ps = ps_pool.tile([2 * ROWS_B, 2 * CHUNK], f32, tag="ps")
                    for k in range(kn):
                        nc.tensor.matmul(
                            ps[base : base + ROWS_B, k * CHUNK : (k + 1) * CHUNK],
                            lhsT_b,
                            xq[:, (k0 + k) * CHUNK : (k0 + k + 1) * CHUNK],
                            start=True,
                            stop=True,
                        )
                    copy(
                        cnt,
                        o_sb[base : base + ROWS_B, k0 * CHUNK : (k0 + kn) * CHUNK],
                        ps[base : base + ROWS_B, : kn * CHUNK],
                    )
                    cnt += 1
                nc.sync.dma_start(
                    out[ROWS_A:, cb * CHUNK : (cb + nb) * CHUNK],
                    o_sb[base : base + ROWS_B, : nb * CHUNK],
                )
    nc.finalize()
    return nc


def _get_module():
    if "nc" not in _COMPILED:
        _install_ntff_hook_shim()
        _COMPILED["nc"] = _build_module()
    return _COMPILED["nc"]


def kernel(x, t, M, M0, b):
    from concourse.bass_utils import run_bass_kernel_spmd

    nc = _get_module()

    bhl = _build_bhl(np.asarray(t), np.asarray(M), np.asarray(M0), np.asarray(b))
    x = np.ascontiguousarray(x, dtype=np.float32)

    in_maps = []
    for c in range(N_CORES):
        xs = np.zeros((N_PAD, M_SIZE), dtype=np.float32)
        xs[:N_SHARD] = x[c * N_SHARD : (c + 1) * N_SHARD]
        xt = xs.reshape(N_CHUNKS, CHUNK, M_SIZE).transpose(0, 2, 1)  # (123,3,512)
        xh = xt.astype(np.float16).transpose(1, 0, 2).reshape(M_SIZE, N_PAD)
        xl = (
            (xt - xt.astype(np.float16).astype(np.float32))
            .astype(np.float16)
            .transpose(1, 0, 2)
            .reshape(M_SIZE, N_PAD)
        )
        # rows: [xh; xh; xl; xl] to pair with the [Bh; Bl; Bh; Bl] stationary
        x6 = np.concatenate([xh, xh, xl, xl], axis=0)  # (12, N_PAD)
        in_maps.append({"x6": x6, "bhl": bhl})

    trace = bool(os.environ.get("AFFINE_KERNEL_TRACE"))
    res = run_bass_kernel_spmd(
        nc, in_maps, core_ids=list(range(N_CORES)), trace=trace
    )
    if trace:
        kernel.last_result = res

    out = np.empty((T_STEPS, N_POINTS, M_SIZE), dtype=np.float32)
    for c in range(N_CORES):
        r = res.results[c]["out"]  # (192, 62976), row m = t*3 + i
        out[:, c * N_SHARD : (c + 1) * N_SHARD, :] = r.reshape(
            T_STEPS, M_SIZE, N_PAD
        ).transpose(0, 2, 1)[:, :N_SHARD, :]
    return out


# revision 29
# speedup vs baseline: 1.3097x; 1.3097x over previous
"""AffineFlow Trainium2 kernel.

Computes out[t, n, i] = sum_j expm(t_k * A)[i, j] * x_pad[n, j] for the
4x4 homogeneous A built from (M, M0, b), with x_pad[:, 3] = 0 and the
homogeneous output row dropped.  Equivalently out[t] = x @ B_t^T with
B_t = expm(t_k * A)[:3, :3].

Distribution: data-parallel over the N=500000 points axis across 8
NeuronCores.  The tiny per-t 3x3 maps B_t are computed host-side (64
4x4 expm's, microseconds) and replicated to every core; each core
computes the 192 x 62976 result for its 62500-point shard on the
TensorEngine and streams the 48 MB to HBM in a planar (t,i)-major
layout (the only layout the PE/PSUM path can produce without an
on-chip transpose).  The host gather permutes each shard back to the
(t, n, i) layout while unsharding.

Precision: fp32 matmul on trn2 PE runs in a 2x-slower LOW_HIGH
emulation with the fp32 moving operand streaming at ~2.5 cycles/col.
Instead we split both operands into fp16 hi+lo pairs (exact to ~2^-22)
and exploit the tiny contraction dim: the stationary is the K=12 block
[Bh; Bl; Bh; Bl] against the moving block [xh; xh; xl; xl], so a
single fp16-rate stream per chunk accumulates the exact expansion
Bh xh + Bl xh + Bh xl + Bl xl = B x in fp32 PSUM.  Max abs error vs
the fp32 reference is ~1e-6.
"""

import os
import sys

import numpy as np

# ---------------------------------------------------------------------------
# Problem constants (hardcoded per the self-containment contract)
# ---------------------------------------------------------------------------
N_POINTS = 500000
T_STEPS = 64
M_SIZE = 3
N_CORES = 8
N_SHARD = N_POINTS // N_CORES  # 62500
CHUNK = 512  # one PSUM bank of fp32
N_CHUNKS = 123  # ceil(62500 / 512) -> padded shard of 62976 points
N_PAD = N_CHUNKS * CHUNK  # 62976
N_OUT = N_SHARD  # stored columns per core (padding columns not written)
ROWS = T_STEPS * M_SIZE  # 192 output rows, split 128 ("A") + 64 ("B")
ROWS_A = 128
ROWS_B = ROWS - ROWS_A  # 64; two chunks packed per 128-partition tile
XBATCH = 8  # chunks per x-load / out-store DMA


def _install_ntff_hook_shim():
    """antenv.axon_hooks is missing in this image; trace=True (or a
    BASS_TRACE=1 env from a harness) would crash run_bass_kernel_spmd
    without it.  Register the hook via the boot module's ctypes factory."""
    try:
        import antenv.axon_hooks  # noqa: F401

        return
    except ImportError:
        pass
    import types

    mod = types.ModuleType("antenv.axon_hooks")
    mod._hook = None
    mod.set_axon_ntff_profile_hook = lambda h: setattr(mod, "_hook", h)
    mod.get_axon_ntff_profile_hook = lambda: mod._hook
    sys.modules["antenv.axon_hooks"] = mod
    try:
        import antenv

        antenv.axon_hooks = mod
    except ImportError:
        pass
    try:
        from trn_agent_boot.trn_boot import _ntff_profile_via_ctypes

        hook = _ntff_profile_via_ctypes("/opt/axon/libaxon_pjrt.so")
        if hook is not None:
            mod._hook = hook
    except Exception:
        pass


def _expm(mat: np.ndarray) -> np.ndarray:
    """Matrix exponential via scaling-and-squaring + Taylor (float64).
    Exact to machine precision for the tiny well-conditioned matrices here."""
    mat = mat.astype(np.float64)
    nrm = np.linalg.norm(mat, 1)
    s = max(0, int(np.ceil(np.log2(max(nrm, 1e-300) / 0.25)))) if nrm > 0.25 else 0
    a = mat / (2.0**s)
    n = mat.shape[0]
    out = np.eye(n) + a
    term = a.copy()
    for k in range(2, 24):
        term = term @ a / k
        out += term
        if np.abs(term).max() < 1e-18:
            break
    for _ in range(s):
        out = out @ out
    return out


def _build_bhl(t: np.ndarray, M: np.ndarray, M0: np.ndarray, b: np.ndarray):
    """Stationary layout: bhl[j, h*96 + tl*3 + i] = B[h*32+tl, i, j] split
    into an fp16 hi/lo pair stacked along K -> shape (6, 192) fp16."""
    A = np.zeros((M_SIZE + 1, M_SIZE + 1), dtype=np.float64)
    A[:M_SIZE, :M_SIZE] = M.astype(np.float64) + M0.astype(np.float64)
    A[:M_SIZE, M_SIZE] = b.astype(np.float64)
    B = np.stack([_expm(float(tk) * A)[:M_SIZE, :M_SIZE] for tk in t])  # (64,3,3)
    bt = B.transpose(2, 0, 1).reshape(M_SIZE, T_STEPS * M_SIZE)  # (3, 192) f64
    bh = bt.astype(np.float16)
    bl = (bt - bh.astype(np.float64)).astype(np.float16)
    return np.ascontiguousarray(np.concatenate([bh, bl, bh, bl], axis=0))  # (12, 192)


_COMPILED = {}


def _build_module():
    from concourse import bacc, mybir
    import concourse.tile as tile

    nc = bacc.Bacc(
        "TRN2",
        target_bir_lowering=False,
        debug=False,
        enable_asserts=False,
        num_devices=N_CORES,
        num_swdge_queues=4,
    )
    f32 = mybir.dt.float32
    f16 = mybir.dt.float16
    # partition-major: rows 0-5 = [xh; xh] per j, rows 6-11 = [xl; xl]
    x6 = nc.dram_tensor(
        "x6", [4 * M_SIZE, N_CHUNKS * CHUNK], f16, kind="ExternalInput"
    )
    bhl = nc.dram_tensor("bhl", [4 * M_SIZE, ROWS], f16, kind="ExternalInput")
    out = nc.dram_tensor("out", [ROWS, N_OUT], f32, kind="ExternalOutput")

    # block schedule: small first block so the store pipeline starts early
    blocks = []
    cb = 0
    first = True
    while cb < N_CHUNKS:
        nb = min(2 if first else XBATCH, N_CHUNKS - cb)
        blocks.append((cb, nb))
        cb += nb
        first = False

    with tile.TileContext(nc) as tc:
        with (
            tc.tile_pool(name="bt", bufs=1) as bt_pool,
            tc.tile_pool(name="x", bufs=6) as x_pool,
            tc.tile_pool(name="ps", bufs=4, space="PSUM") as ps_pool,
            tc.tile_pool(name="o", bufs=5) as o_pool,
            tc.tile_pool(name="ob", bufs=3) as ob_pool,
        ):
            bhl_sb = bt_pool.tile([4 * M_SIZE, ROWS], f16)
            nc.sync.dma_start(bhl_sb[:], bhl[:])
            lhsT_a = bhl_sb[:, :ROWS_A]  # [12, 128]
            lhsT_b = bhl_sb[:, ROWS_A:]  # [12, 64]

            def copy(cnt, dst, src):
                # ACT-weighted 2:1 split (ACT is faster for wide fp32 copies)
                if cnt % 3 < 2:
                    nc.scalar.copy(dst, src)
                else:
                    nc.vector.tensor_copy(dst, src)

            cnt = 0
            # --- group A: output rows 0..127, full-partition tiles ---
            for cb, nb in blocks:
                xq = x_pool.tile([4 * M_SIZE, XBATCH * CHUNK], f16, tag="xq")
                # x-loads go via gpsimd/SWDGE (4 queues) so they don't
                # FIFO-block the out-stores on the SP HWDGE ring
                nc.gpsimd.dma_start(
                    xq[:, : nb * CHUNK],
                    x6[:, cb * CHUNK : (cb + nb) * CHUNK],
                )
                w = min(nb * CHUNK, N_OUT - cb * CHUNK)
                o_sb = o_pool.tile([ROWS_A, XBATCH * CHUNK], f32, tag="osb")
                for s in range((nb + 1) // 2):
                    k0 = 2 * s
                    kn = min(2, nb - k0)  # chunks in this psum tile
                    ps = ps_pool.tile([ROWS_A, 2 * CHUNK], f32, tag="ps")
                    for k in range(kn):
                        nc.tensor.matmul(
                            ps[:, k * CHUNK : (k + 1) * CHUNK],
                            lhsT_a,
                            xq[:, (k0 + k) * CHUNK : (k0 + k + 1) * CHUNK],
                            start=True,
                            stop=True,
                        )
                    copy(
                        cnt,
                        o_sb[:, k0 * CHUNK : (k0 + kn) * CHUNK],
                        ps[:, : kn * CHUNK],
                    )
                    cnt += 1
                nc.sync.dma_start(
                    out[:ROWS_A, cb * CHUNK : cb * CHUNK + w],
                    o_sb[:, :w],
                )
            # --- group B: output rows 128..191; [64, .] tiles at alternating
            # partition base per block so SBUF-port load stays balanced ---
            for bi, (cb, nb) in enumerate(blocks):
                base = ROWS_B * (bi % 2)
                xq = x_pool.tile([4 * M_SIZE, XBATCH * CHUNK], f16, tag="xq")
                nc.gpsimd.dma_start(
                    xq[:, : nb * CHUNK],
                    x6[:, cb * CHUNK : (cb + nb) * CHUNK],
                )
                w = min(nb * CHUNK, N_OUT - cb * CHUNK)
                o_sb = ob_pool.tile([2 * ROWS_B, XBATCH * CHUNK], f32, tag="osbB")
                for s in range((nb + 1) // 2):
                    k0 = 2 * s
                    kn = min(2, nb - k0)
                    ps = ps_pool.tile([2 * ROWS_B, 2 * CHUNK], f32, tag="ps")
                    for k in range(kn):
                        nc.tensor.matmul(
                            ps[base : base + ROWS_B, k * CHUNK : (k + 1) * CHUNK],
                            lhsT_b,
                            xq[:, (k0 + k) * CHUNK : (k0 + k + 1) * CHUNK],
                            start=True,
                            stop=True,
                        )
                    copy(
                        cnt,
                        o_sb[base : base + ROWS_B, k0 * CHUNK : (k0 + kn) * CHUNK],
                        ps[base : base + ROWS_B, : kn * CHUNK],
                    )
                    cnt += 1
                nc.sync.dma_start(
                    out[ROWS_A:, cb * CHUNK : cb * CHUNK + w],
                    o_sb[base : base + ROWS_B, :w],
                )
    nc.finalize()
    return nc


def _get_module():
    if "nc" not in _COMPILED:
        _install_ntff_hook_shim()
        _COMPILED["nc"] = _build_module()
    return _COMPILED["nc"]


def kernel(x, t, M, M0, b):
    from concourse.bass_utils import run_bass_kernel_spmd

    nc = _get_module()

    bhl = _build_bhl(np.asarray(t), np.asarray(M), np.asarray(M0), np.asarray(b))
    x = np.ascontiguousarray(x, dtype=np.float32)

    in_maps = []
    for c in range(N_CORES):
        xs = np.zeros((N_PAD, M_SIZE), dtype=np.float32)
        xs[:N_SHARD] = x[c * N_SHARD : (c + 1) * N_SHARD]
        xt = xs.reshape(N_CHUNKS, CHUNK, M_SIZE).transpose(0, 2, 1)  # (123,3,512)
        xh = xt.astype(np.float16).transpose(1, 0, 2).reshape(M_SIZE, N_PAD)
        xl = (
            (xt - xt.astype(np.float16).astype(np.float32))
            .astype(np.float16)
            .transpose(1, 0, 2)
            .reshape(M_SIZE, N_PAD)
        )
        # rows: [xh; xh; xl; xl] to pair with the [Bh; Bl; Bh; Bl] stationary
        x6 = np.concatenate([xh, xh, xl, xl], axis=0)  # (12, N_PAD)
        in_maps.append({"x6": x6, "bhl": bhl})

    trace = bool(os.environ.get("AFFINE_KERNEL_TRACE"))
    res = run_bass_kernel_spmd(
        nc, in_maps, core_ids=list(range(N_CORES)), trace=trace
    )
    if trace:
        kernel.last_result = res

    out = np.empty((T_STEPS, N_POINTS, M_SIZE), dtype=np.float32)
    for c in range(N_CORES):
        r = res.results[c]["out"]  # (192, 62500), row m = t*3 + i
        out[:, c * N_SHARD : (c + 1) * N_SHARD, :] = r.reshape(
            T_STEPS, M_SIZE, N_OUT
        ).transpose(0, 2, 1)
    return out


# revision 30
# speedup vs baseline: 1.3657x; 1.0428x over previous
"""AffineFlow Trainium2 kernel.

Computes out[t, n, i] = sum_j expm(t_k * A)[i, j] * x_pad[n, j] for the
4x4 homogeneous A built from (M, M0, b), with x_pad[:, 3] = 0 and the
homogeneous output row dropped.  Equivalently out[t] = x @ B_t^T with
B_t = expm(t_k * A)[:3, :3].

Distribution: data-parallel over the N=500000 points axis across 8
NeuronCores.  The tiny per-t 3x3 maps B_t are computed host-side (64
4x4 expm's, microseconds) and replicated to every core; each core
computes the 192 x 62976 result for its 62500-point shard on the
TensorEngine and streams the 48 MB to HBM in a planar (t,i)-major
layout (the only layout the PE/PSUM path can produce without an
on-chip transpose).  The host gather permutes each shard back to the
(t, n, i) layout while unsharding.

Precision: fp32 matmul on trn2 PE runs in a 2x-slower LOW_HIGH
emulation with the fp32 moving operand streaming at ~2.5 cycles/col.
Instead we split both operands into fp16 hi+lo pairs (exact to ~2^-22)
and exploit the tiny contraction dim: the stationary is the K=12 block
[Bh; Bl; Bh; Bl] against the moving block [xh; xh; xl; xl], so a
single fp16-rate stream per chunk accumulates the exact expansion
Bh xh + Bl xh + Bh xl + Bl xl = B x in fp32 PSUM.  Max abs error vs
the fp32 reference is ~1e-6.
"""

import os
import sys

import numpy as np

# ---------------------------------------------------------------------------
# Problem constants (hardcoded per the self-containment contract)
# ---------------------------------------------------------------------------
N_POINTS = 500000
T_STEPS = 64
M_SIZE = 3
N_CORES = 8
N_SHARD = N_POINTS // N_CORES  # 62500
CHUNK = 512  # one PSUM bank of fp32
N_CHUNKS = 123  # ceil(62500 / 512) -> padded shard of 62976 points
N_PAD = N_CHUNKS * CHUNK  # 62976
N_OUT = N_SHARD  # stored columns per core (padding columns not written)
ROWS = T_STEPS * M_SIZE  # 192 output rows, split 128 ("A") + 64 ("B")
ROWS_A = 128
ROWS_B = ROWS - ROWS_A  # 64; two chunks packed per 128-partition tile
XBATCH = 8  # chunks per x-load / out-store DMA


def _install_ntff_hook_shim():
    """antenv.axon_hooks is missing in this image; trace=True (or a
    BASS_TRACE=1 env from a harness) would crash run_bass_kernel_spmd
    without it.  Register the hook via the boot module's ctypes factory."""
    try:
        import antenv.axon_hooks  # noqa: F401

        return
    except ImportError:
        pass
    import types

    mod = types.ModuleType("antenv.axon_hooks")
    mod._hook = None
    mod.set_axon_ntff_profile_hook = lambda h: setattr(mod, "_hook", h)
    mod.get_axon_ntff_profile_hook = lambda: mod._hook
    sys.modules["antenv.axon_hooks"] = mod
    try:
        import antenv

        antenv.axon_hooks = mod
    except ImportError:
        pass
    try:
        from trn_agent_boot.trn_boot import _ntff_profile_via_ctypes

        hook = _ntff_profile_via_ctypes("/opt/axon/libaxon_pjrt.so")
        if hook is not None:
            mod._hook = hook
    except Exception:
        pass


def _expm(mat: np.ndarray) -> np.ndarray:
    """Matrix exponential via scaling-and-squaring + Taylor (float64).
    Exact to machine precision for the tiny well-conditioned matrices here."""
    mat = mat.astype(np.float64)
    nrm = np.linalg.norm(mat, 1)
    s = max(0, int(np.ceil(np.log2(max(nrm, 1e-300) / 0.25)))) if nrm > 0.25 else 0
    a = mat / (2.0**s)
    n = mat.shape[0]
    out = np.eye(n) + a
    term = a.copy()
    for k in range(2, 24):
        term = term @ a / k
        out += term
        if np.abs(term).max() < 1e-18:
            break
    for _ in range(s):
        out = out @ out
    return out


def _build_bhl(t: np.ndarray, M: np.ndarray, M0: np.ndarray, b: np.ndarray):
    """Stationary layout: bhl[j, h*96 + tl*3 + i] = B[h*32+tl, i, j] split
    into an fp16 hi/lo pair stacked along K -> shape (6, 192) fp16."""
    A = np.zeros((M_SIZE + 1, M_SIZE + 1), dtype=np.float64)
    A[:M_SIZE, :M_SIZE] = M.astype(np.float64) + M0.astype(np.float64)
    A[:M_SIZE, M_SIZE] = b.astype(np.float64)
    B = np.stack([_expm(float(tk) * A)[:M_SIZE, :M_SIZE] for tk in t])  # (64,3,3)
    bt = B.transpose(2, 0, 1).reshape(M_SIZE, T_STEPS * M_SIZE)  # (3, 192) f64
    bh = bt.astype(np.float16)
    bl = (bt - bh.astype(np.float64)).astype(np.float16)
    return np.ascontiguousarray(np.concatenate([bh, bl, bh, bl], axis=0))  # (12, 192)


_COMPILED = {}


def _build_module():
    from concourse import bacc, mybir
    import concourse.tile as tile

    nc = bacc.Bacc(
        "TRN2",
        target_bir_lowering=False,
        debug=False,
        enable_asserts=False,
        num_devices=N_CORES,
        num_swdge_queues=4,
    )
    f32 = mybir.dt.float32
    f16 = mybir.dt.float16
    # partition-major: rows 0-5 = [xh; xh] per j, rows 6-11 = [xl; xl]
    x6 = nc.dram_tensor(
        "x6", [4 * M_SIZE, N_CHUNKS * CHUNK], f16, kind="ExternalInput"
    )
    bhl = nc.dram_tensor("bhl", [4 * M_SIZE, ROWS], f16, kind="ExternalInput")
    out = nc.dram_tensor("out", [ROWS, N_OUT], f32, kind="ExternalOutput")

    # block schedule: small first block so the store pipeline starts early
    blocks = []
    cb = 0
    first = True
    while cb < N_CHUNKS:
        nb = min(2 if first else XBATCH, N_CHUNKS - cb)
        blocks.append((cb, nb))
        cb += nb
        first = False

    with tile.TileContext(nc) as tc:
        with (
            tc.tile_pool(name="bt", bufs=1) as bt_pool,
            tc.tile_pool(name="x", bufs=4) as x_pool,
            tc.tile_pool(name="ps", bufs=4, space="PSUM") as ps_pool,
            tc.tile_pool(name="o", bufs=5) as o_pool,
        ):
            bhl_sb = bt_pool.tile([4 * M_SIZE, ROWS], f16)
            nc.sync.dma_start(bhl_sb[:], bhl[:])
            lhsT_a = bhl_sb[:, :ROWS_A]  # [12, 128]
            lhsT_b = bhl_sb[:, ROWS_A:]  # [12, 64]

            def copy(cnt, dst, src):
                # ACT-weighted 2:1 split (ACT is faster for wide fp32 copies)
                if cnt % 3 < 2:
                    nc.scalar.copy(dst, src)
                else:
                    nc.vector.tensor_copy(dst, src)

            cnt = 0
            # --- group A: output rows 0..127, full-partition tiles ---
            for cb, nb in blocks:
                xq = x_pool.tile([4 * M_SIZE, XBATCH * CHUNK], f16, tag="xq")
                # x-loads go via gpsimd/SWDGE (4 queues) so they don't
                # FIFO-block the out-stores on the SP HWDGE ring
                nc.gpsimd.dma_start(
                    xq[:, : nb * CHUNK],
                    x6[:, cb * CHUNK : (cb + nb) * CHUNK],
                )
                w = min(nb * CHUNK, N_OUT - cb * CHUNK)
                o_sb = o_pool.tile([ROWS_A, XBATCH * CHUNK], f32, tag="osb")
                for s in range((nb + 1) // 2):
                    k0 = 2 * s
                    kn = min(2, nb - k0)  # chunks in this psum tile
                    ps = ps_pool.tile([ROWS_A, 2 * CHUNK], f32, tag="ps")
                    for k in range(kn):
                        nc.tensor.matmul(
                            ps[:, k * CHUNK : (k + 1) * CHUNK],
                            lhsT_a,
                            xq[:, (k0 + k) * CHUNK : (k0 + k + 1) * CHUNK],
                            start=True,
                            stop=True,
                        )
                    copy(
                        cnt,
                        o_sb[:, k0 * CHUNK : (k0 + kn) * CHUNK],
                        ps[:, : kn * CHUNK],
                    )
                    cnt += 1
                nc.sync.dma_start(
                    out[:ROWS_A, cb * CHUNK : cb * CHUNK + w],
                    o_sb[:, :w],
                )
            # --- group B: output rows 128..191; [64, .] tiles at alternating
            # partition base per block so SBUF-port load stays balanced ---
            for bi, (cb, nb) in enumerate(blocks):
                base = ROWS_B * (bi % 2)
                xq = x_pool.tile([4 * M_SIZE, XBATCH * CHUNK], f16, tag="xq")
                nc.gpsimd.dma_start(
                    xq[:, : nb * CHUNK],
                    x6[:, cb * CHUNK : (cb + nb) * CHUNK],
                )
                w = min(nb * CHUNK, N_OUT - cb * CHUNK)
                o_sb = o_pool.tile([2 * ROWS_B, XBATCH * CHUNK], f32, tag="osbB")
                for s in range((nb + 1) // 2):
                    k0 = 2 * s
                    kn = min(2, nb - k0)
                    ps = ps_pool.tile([2 * ROWS_B, 2 * CHUNK], f32, tag="ps")
                    for k in range(kn):
                        nc.tensor.matmul(
                            ps[base : base + ROWS_B, k * CHUNK : (k + 1) * CHUNK],
                            lhsT_b,
                            xq[:, (k0 + k) * CHUNK : (k0 + k + 1) * CHUNK],
                            start=True,
                            stop=True,
                        )
                    copy(
                        cnt,
                        o_sb[base : base + ROWS_B, k0 * CHUNK : (k0 + kn) * CHUNK],
                        ps[base : base + ROWS_B, : kn * CHUNK],
                    )
                    cnt += 1
                nc.sync.dma_start(
                    out[ROWS_A:, cb * CHUNK : cb * CHUNK + w],
                    o_sb[base : base + ROWS_B, :w],
                )
    nc.finalize()
    return nc


def _get_module():
    if "nc" not in _COMPILED:
        _install_ntff_hook_shim()
        _COMPILED["nc"] = _build_module()
    return _COMPILED["nc"]


def kernel(x, t, M, M0, b):
    from concourse.bass_utils import run_bass_kernel_spmd

    nc = _get_module()

    bhl = _build_bhl(np.asarray(t), np.asarray(M), np.asarray(M0), np.asarray(b))
    x = np.ascontiguousarray(x, dtype=np.float32)

    in_maps = []
    for c in range(N_CORES):
        xs = np.zeros((N_PAD, M_SIZE), dtype=np.float32)
        xs[:N_SHARD] = x[c * N_SHARD : (c + 1) * N_SHARD]
        xt = xs.reshape(N_CHUNKS, CHUNK, M_SIZE).transpose(0, 2, 1)  # (123,3,512)
        xh = xt.astype(np.float16).transpose(1, 0, 2).reshape(M_SIZE, N_PAD)
        xl = (
            (xt - xt.astype(np.float16).astype(np.float32))
            .astype(np.float16)
            .transpose(1, 0, 2)
            .reshape(M_SIZE, N_PAD)
        )
        # rows: [xh; xh; xl; xl] to pair with the [Bh; Bl; Bh; Bl] stationary
        x6 = np.concatenate([xh, xh, xl, xl], axis=0)  # (12, N_PAD)
        in_maps.append({"x6": x6, "bhl": bhl})

    trace = bool(os.environ.get("AFFINE_KERNEL_TRACE"))
    res = run_bass_kernel_spmd(
        nc, in_maps, core_ids=list(range(N_CORES)), trace=trace
    )
    if trace:
        kernel.last_result = res

    out = np.empty((T_STEPS, N_POINTS, M_SIZE), dtype=np.float32)
    for c in range(N_CORES):
        r = res.results[c]["out"]  # (192, 62500), row m = t*3 + i
        out[:, c * N_SHARD : (c + 1) * N_SHARD, :] = r.reshape(
            T_STEPS, M_SIZE, N_OUT
        ).transpose(0, 2, 1)
    return out


# revision 35
# speedup vs baseline: 1.4056x; 1.0292x over previous
"""AffineFlow Trainium2 kernel.

Computes out[t, n, i] = sum_j expm(t_k * A)[i, j] * x_pad[n, j] for the
4x4 homogeneous A built from (M, M0, b), with x_pad[:, 3] = 0 and the
homogeneous output row dropped.  Equivalently out[t] = x @ B_t^T with
B_t = expm(t_k * A)[:3, :3].

Distribution: data-parallel over the N=500000 points axis across 8
NeuronCores.  The tiny per-t 3x3 maps B_t are computed host-side (64
4x4 expm's, microseconds) and replicated to every core; each core
computes the 192 x 62976 result for its 62500-point shard on the
TensorEngine and streams the 48 MB to HBM in a planar (t,i)-major
layout (the only layout the PE/PSUM path can produce without an
on-chip transpose).  The host gather permutes each shard back to the
(t, n, i) layout while unsharding.

Precision: fp32 matmul on trn2 PE runs in a 2x-slower LOW_HIGH
emulation with the fp32 moving operand streaming at ~2.5 cycles/col.
Instead we split both operands into fp16 hi+lo pairs (exact to ~2^-22)
and exploit the tiny contraction dim: the stationary is the K=12 block
[Bh; Bl; Bh; Bl] against the moving block [xh; xh; xl; xl], so a
single fp16-rate stream per chunk accumulates the exact expansion
Bh xh + Bl xh + Bh xl + Bl xl = B x in fp32 PSUM.  Max abs error vs
the fp32 reference is ~1e-6.
"""

import os
import sys

import numpy as np

# ---------------------------------------------------------------------------
# Problem constants (hardcoded per the self-containment contract)
# ---------------------------------------------------------------------------
N_POINTS = 500000
T_STEPS = 64
M_SIZE = 3
N_CORES = 8
N_SHARD = N_POINTS // N_CORES  # 62500
CHUNK = 512  # one PSUM bank of fp32
N_CHUNKS = 123  # ceil(62500 / 512) -> padded shard of 62976 points
N_PAD = N_CHUNKS * CHUNK  # 62976
N_OUT = N_SHARD  # stored columns per core (padding columns not written)
ROWS = T_STEPS * M_SIZE  # 192 output rows, split 128 ("A") + 64 ("B")
ROWS_A = 128
ROWS_B = ROWS - ROWS_A  # 64; two chunks packed per 128-partition tile
XBATCH = 8  # chunks per x-load / out-store DMA


def _install_ntff_hook_shim():
    """antenv.axon_hooks is missing in this image; trace=True (or a
    BASS_TRACE=1 env from a harness) would crash run_bass_kernel_spmd
    without it.  Register the hook via the boot module's ctypes factory."""
    try:
        import antenv.axon_hooks  # noqa: F401

        return
    except ImportError:
        pass
    import types

    mod = types.ModuleType("antenv.axon_hooks")
    mod._hook = None
    mod.set_axon_ntff_profile_hook = lambda h: setattr(mod, "_hook", h)
    mod.get_axon_ntff_profile_hook = lambda: mod._hook
    sys.modules["antenv.axon_hooks"] = mod
    try:
        import antenv

        antenv.axon_hooks = mod
    except ImportError:
        pass
    try:
        from trn_agent_boot.trn_boot import _ntff_profile_via_ctypes

        hook = _ntff_profile_via_ctypes("/opt/axon/libaxon_pjrt.so")
        if hook is not None:
            mod._hook = hook
    except Exception:
        pass


def _expm(mat: np.ndarray) -> np.ndarray:
    """Matrix exponential via scaling-and-squaring + Taylor (float64).
    Exact to machine precision for the tiny well-conditioned matrices here."""
    mat = mat.astype(np.float64)
    nrm = np.linalg.norm(mat, 1)
    s = max(0, int(np.ceil(np.log2(max(nrm, 1e-300) / 0.25)))) if nrm > 0.25 else 0
    a = mat / (2.0**s)
    n = mat.shape[0]
    out = np.eye(n) + a
    term = a.copy()
    for k in range(2, 24):
        term = term @ a / k
        out += term
        if np.abs(term).max() < 1e-18:
            break
    for _ in range(s):
        out = out @ out
    return out


def _build_bhl(t: np.ndarray, M: np.ndarray, M0: np.ndarray, b: np.ndarray):
    """Stationary layout: column m = t*3 + i holds B[t, i, :] split into an
    fp16 hi/lo pair stacked [Bh; Bl; Bh; Bl] along K -> shape (12, 192)."""
    A = np.zeros((M_SIZE + 1, M_SIZE + 1), dtype=np.float64)
    A[:M_SIZE, :M_SIZE] = M.astype(np.float64) + M0.astype(np.float64)
    A[:M_SIZE, M_SIZE] = b.astype(np.float64)
    B = np.stack([_expm(float(tk) * A)[:M_SIZE, :M_SIZE] for tk in t])  # (64,3,3)
    bt = B.transpose(2, 0, 1).reshape(M_SIZE, T_STEPS * M_SIZE)  # (3, 192) f64
    bh = bt.astype(np.float16)
    bl = (bt - bh.astype(np.float64)).astype(np.float16)
    return np.ascontiguousarray(np.concatenate([bh, bl, bh, bl], axis=0))  # (12, 192)


_COMPILED = {}


def _build_module():
    from concourse import bacc, mybir
    import concourse.tile as tile

    nc = bacc.Bacc(
        "TRN2",
        target_bir_lowering=False,
        debug=False,
        enable_asserts=False,
        num_devices=N_CORES,
        num_swdge_queues=4,
    )
    f32 = mybir.dt.float32
    f16 = mybir.dt.float16
    # partition-major: rows 0-5 = [xh; xh] per j, rows 6-11 = [xl; xl]
    x6 = nc.dram_tensor(
        "x6", [4 * M_SIZE, N_CHUNKS * CHUNK], f16, kind="ExternalInput"
    )
    bhl = nc.dram_tensor("bhl", [4 * M_SIZE, ROWS], f16, kind="ExternalInput")
    out = nc.dram_tensor("out", [ROWS, N_OUT], f32, kind="ExternalOutput")

    # block schedule: small first block so the store pipeline starts early
    blocks = []
    cb = 0
    first = True
    while cb < N_CHUNKS:
        nb = min(2 if first else XBATCH, N_CHUNKS - cb)
        blocks.append((cb, nb))
        cb += nb
        first = False

    with tile.TileContext(nc) as tc:
        with (
            tc.tile_pool(name="bt", bufs=1) as bt_pool,
            tc.tile_pool(name="x", bufs=4) as x_pool,
            tc.tile_pool(name="ps", bufs=4, space="PSUM") as ps_pool,
            tc.tile_pool(name="o", bufs=5) as o_pool,
        ):
            bhl_sb = bt_pool.tile([4 * M_SIZE, ROWS], f16)
            nc.sync.dma_start(bhl_sb[:], bhl[:])
            lhsT_a = bhl_sb[:, :ROWS_A]  # [12, 128]
            lhsT_b = bhl_sb[:, ROWS_A:]  # [12, 64]

            def copy(cnt, dst, src):
                # ACT-weighted 3:2 split (ACT is faster for wide fp32 copies)
                if cnt % 5 < 3:
                    nc.scalar.copy(dst, src)
                else:
                    nc.vector.tensor_copy(dst, src)

            cnt = 0
            # --- group A: output rows 0..127, full-partition tiles ---
            for cb, nb in blocks:
                xq = x_pool.tile([4 * M_SIZE, XBATCH * CHUNK], f16, tag="xq")
                # x-loads go via gpsimd/SWDGE (4 queues) so they don't
                # FIFO-block the out-stores on the SP HWDGE ring
                nc.gpsimd.dma_start(
                    xq[:, : nb * CHUNK],
                    x6[:, cb * CHUNK : (cb + nb) * CHUNK],
                )
                w = min(nb * CHUNK, N_OUT - cb * CHUNK)
                o_sb = o_pool.tile([ROWS_A, XBATCH * CHUNK], f32, tag="osb")
                for s in range((nb + 1) // 2):
                    k0 = 2 * s
                    kn = min(2, nb - k0)  # chunks in this psum tile
                    ps = ps_pool.tile([ROWS_A, 2 * CHUNK], f32, tag="ps")
                    for k in range(kn):
                        nc.tensor.matmul(
                            ps[:, k * CHUNK : (k + 1) * CHUNK],
                            lhsT_a,
                            xq[:, (k0 + k) * CHUNK : (k0 + k + 1) * CHUNK],
                            start=True,
                            stop=True,
                        )
                    copy(
                        cnt,
                        o_sb[:, k0 * CHUNK : (k0 + kn) * CHUNK],
                        ps[:, : kn * CHUNK],
                    )
                    cnt += 1
                nc.sync.dma_start(
                    out[:ROWS_A, cb * CHUNK : cb * CHUNK + w],
                    o_sb[:, :w],
                )
            # --- group B: output rows 128..191; [64, .] tiles at alternating
            # partition base per block so SBUF-port load stays balanced ---
            for bi, (cb, nb) in enumerate(blocks):
                base = ROWS_B * (bi % 2)
                xq = x_pool.tile([4 * M_SIZE, XBATCH * CHUNK], f16, tag="xq")
                nc.gpsimd.dma_start(
                    xq[:, : nb * CHUNK],
                    x6[:, cb * CHUNK : (cb + nb) * CHUNK],
                )
                w = min(nb * CHUNK, N_OUT - cb * CHUNK)
                o_sb = o_pool.tile([2 * ROWS_B, XBATCH * CHUNK], f32, tag="osbB")
                for s in range((nb + 1) // 2):
                    k0 = 2 * s
                    kn = min(2, nb - k0)
                    ps = ps_pool.tile([2 * ROWS_B, 2 * CHUNK], f32, tag="ps")
                    for k in range(kn):
                        nc.tensor.matmul(
                            ps[base : base + ROWS_B, k * CHUNK : (k + 1) * CHUNK],
                            lhsT_b,
                            xq[:, (k0 + k) * CHUNK : (k0 + k + 1) * CHUNK],
                            start=True,
                            stop=True,
                        )
                    copy(
                        cnt,
                        o_sb[base : base + ROWS_B, k0 * CHUNK : (k0 + kn) * CHUNK],
                        ps[base : base + ROWS_B, : kn * CHUNK],
                    )
                    cnt += 1
                nc.sync.dma_start(
                    out[ROWS_A:, cb * CHUNK : cb * CHUNK + w],
                    o_sb[base : base + ROWS_B, :w],
                )
    nc.finalize()
    return nc


def _get_module():
    if "nc" not in _COMPILED:
        _install_ntff_hook_shim()
        _COMPILED["nc"] = _build_module()
    return _COMPILED["nc"]


def kernel(x, t, M, M0, b):
    from concourse.bass_utils import run_bass_kernel_spmd

    nc = _get_module()

    bhl = _build_bhl(np.asarray(t), np.asarray(M), np.asarray(M0), np.asarray(b))
    x = np.ascontiguousarray(x, dtype=np.float32)

    in_maps = []
    for c in range(N_CORES):
        xs = np.zeros((N_PAD, M_SIZE), dtype=np.float32)
        xs[:N_SHARD] = x[c * N_SHARD : (c + 1) * N_SHARD]
        xt = xs.reshape(N_CHUNKS, CHUNK, M_SIZE).transpose(0, 2, 1)  # (123,3,512)
        xh = xt.astype(np.float16).transpose(1, 0, 2).reshape(M_SIZE, N_PAD)
        xl = (
            (xt - xt.astype(np.float16).astype(np.float32))
            .astype(np.float16)
            .transpose(1, 0, 2)
            .reshape(M_SIZE, N_PAD)
        )
        # rows: [xh; xh; xl; xl] to pair with the [Bh; Bl; Bh; Bl] stationary
        x6 = np.concatenate([xh, xh, xl, xl], axis=0)  # (12, N_PAD)
        in_maps.append({"x6": x6, "bhl": bhl})

    trace = bool(os.environ.get("AFFINE_KERNEL_TRACE"))
    res = run_bass_kernel_spmd(
        nc, in_maps, core_ids=list(range(N_CORES)), trace=trace
    )
    if trace:
        kernel.last_result = res

    out = np.empty((T_STEPS, N_POINTS, M_SIZE), dtype=np.float32)
    for c in range(N_CORES):
        r = res.results[c]["out"]  # (192, 62500), row m = t*3 + i
        out[:, c * N_SHARD : (c + 1) * N_SHARD, :] = r.reshape(
            T_STEPS, M_SIZE, N_OUT
        ).transpose(0, 2, 1)
    return out
